# revision 17
# baseline (speedup 1.0000x reference)
"""CrossAttention Trainium2 kernel (8 NeuronCores, SPMD), bf16 compute.

Sharding: data-parallel over batch B=2, tensor-parallel over the 16 heads in
4 groups of 4 heads -> 8 cores, one (batch, head-group) pair each. Each core
computes its 4 heads' Q/K/V projections, masked softmax cross-attention, and
its partial output projection y_g = softmax(q k^T * scale) v @ Wo[:, g].T.
The host sums the 4 partial outputs per batch (the Wo row-split all-reduce,
done at unshard time) and adds the v-bias term Wo @ b_v, which is constant
across rows and factors out of the attention (softmax rows sum to 1).
The k-bias is dropped entirely: (k+b)^T q adds a per-query constant to the
logits, which softmax over keys is invariant to.

Numerics: inputs are cast to bf16 on the host; every matmul runs bf16 x bf16
with fp32 PSUM accumulation; softmax statistics (denominator, reciprocal,
normalization) stay fp32. Output partials are written bf16 and summed fp32
on the host. End-to-end relative error ~3e-3.

Layout: the PE contracts over the partition dim, so activations and weights
arrive contraction-major (pre-transposed on host); every device DMA is a
plain contiguous row load; no transposes on the device.

Attention is computed scores-transposed: ST[m, n] per head so PV contracts
over m directly; the two heads of a pair sit at partitions 0:64 / 64:128 so
their score matmuls run concurrently as PE row-tiles. The softmax
denominator comes free from a ones-column appended to v. exp() is
unnormalized (|s*scale| < ~4); mask zeros are applied multiplicatively
after exp with one wide (free=2048) DVE multiply per (pair, m-tile).

Stages (PE-dense, ACT overlapped):
  Q proj (chunk-major, DMA-paced) ->
  stage 1: scores+exp+mask heads 0,1 with K and V projections on PE slack
  stage 2: PV heads 0,1 interleaved with scores+exp+mask heads 2,3
  stage 3: PV head 2 (normalize heads 0,1 in its shadow), PV head 3
           (normalize head 2), normalize head 3, output projection.
Normalization: the denominator row is broadcast to 64 partitions with a
K=1 ones matmul on the PE (rhs read from the evicted ot_sb row at
partition 64), then DVE reciprocal + scale; odd heads shift into the upper
partition half via SBUF-SBUF DMA.
"""

import os

import numpy as np
import ml_dtypes

import concourse.bass as bass
import concourse.bacc as bacc
import concourse.mybir as mybir
import concourse.tile as tile
from concourse.bass_utils import run_bass_kernel_spmd

DIM = 1024
HEAD_DIM = 64
NUM_HEADS = 16
SCALE = HEAD_DIM**-0.5
B, N, M = 2, 1024, 2048
HPC = 4  # heads per core
E = HPC * HEAD_DIM  # 256: per-core projection width
P = 128
F32 = mybir.dt.float32
BF16 = mybir.dt.bfloat16
CT = DIM // P  # 8 contraction tiles
MT = M // P  # 16 m tiles


def build_program():
    nc = bacc.Bacc("TRN2", target_bir_lowering=False, debug=False, num_devices=8)

    # all activation/weight shards arrive contraction-major (pre-transposed)
    xT_d = nc.dram_tensor("xT", [DIM, N], BF16, kind="ExternalInput").ap()
    ctxT_d = nc.dram_tensor("ctxT", [DIM, M], BF16, kind="ExternalInput").ap()
    maskt_d = nc.dram_tensor("maskt", [M, N], BF16, kind="ExternalInput").ap()
    wqT_d = nc.dram_tensor("wqT", [DIM, E], BF16, kind="ExternalInput").ap()
    wkT_d = nc.dram_tensor("wkT", [DIM, E], BF16, kind="ExternalInput").ap()
    wvT_d = nc.dram_tensor("wvT", [DIM, E], BF16, kind="ExternalInput").ap()
    woT_d = nc.dram_tensor("woT", [E, DIM], BF16, kind="ExternalInput").ap()
    y_d = nc.dram_tensor("y", [N, DIM], BF16, kind="ExternalOutput").ap()

    kdbg = bool(os.environ.get("KDBG"))
    if kdbg:
        otdump_d = nc.dram_tensor(
            "otdump", [HEAD_DIM + 1, HPC, N], F32, kind="ExternalOutput"
        ).ap()

    Exp = mybir.ActivationFunctionType.Exp

    from contextlib import ExitStack

    with tile.TileContext(nc) as tc, ExitStack() as ctx:
        persist = ctx.enter_context(tc.tile_pool(name="persist", bufs=1))
        qT = persist.tile([P, E // P, N], BF16)
        kT = persist.tile([P, E // P, M], BF16)
        # v columns padded to 128 so the PV stationary is a full-width
        # weight load (enables Fast Weight Load; col 64 = ones for the
        # softmax denominator, cols 65:128 = don't-care)
        vaug = persist.tile([P, MT, HPC, P], BF16)
        woT = persist.tile([P, E // P, DIM], BF16)
        otn2 = persist.tile([P, E // P, N], BF16)
        ones_sb = persist.tile([P, HEAD_DIM], F32)
        # rows 0:64 unnormalized attention out, row 64 denominator
        ot_sb = persist.tile([HEAD_DIM + 1, HPC, N], F32)

        # ones column: fill everything; v evictions overwrite cols 0:64
        nc.vector.memset(vaug, 1.0)
        nc.vector.memset(ones_sb, 1.0)

        bwork = ctx.enter_context(tc.tile_pool(name="bwork", bufs=2))
        maskp = ctx.enter_context(tc.tile_pool(name="maskp", bufs=3))
        rbp = ctx.enter_context(tc.tile_pool(name="rbp", bufs=2))

        def emit_scores(spool, sbufs, hp, mt, exmst, mk):
            """scores -> exp for head pair hp at m-tile mt (per n-chunk PSUM
            tiles, double-buffered), then one wide masked multiply."""
            ex = bwork.tile([P, 2, N], BF16, tag="ex", name="ex")
            for chn in range(N // 512):
                st = spool.tile(
                    [P, 2, 512], F32, tag="st", name="st", bufs=sbufs
                )
                for hl in range(2):
                    erow = slice(hl * HEAD_DIM, (hl + 1) * HEAD_DIM)
                    nc.tensor.matmul(
                        st[:, hl, :],
                        lhsT=kT[erow, hp, mt * P : (mt + 1) * P],
                        rhs=qT[erow, hp, chn * 512 : (chn + 1) * 512],
                        start=True,
                        stop=True,
                    )
                nc.scalar.activation(
                    ex[:, :, chn * 512 : (chn + 1) * 512], st, Exp,
                    scale=float(SCALE),
                )
            mkc = bass.AP(mk.tensor, mk.offset, [mk.ap[0], [0, 2], mk.ap[1]])
            nc.vector.tensor_mul(exmst[:, mt, :, :], ex, mkc)

        def emit_pv(ot_ps, hp, mt, exmst):
            for hl in range(2):
                h = hp * 2 + hl
                for chn in range(N // 512):
                    nc.tensor.matmul(
                        ot_ps[hl * 2 + chn],
                        lhsT=vaug[:, mt, h, :],
                        rhs=exmst[:, mt, hl, chn * 512 : (chn + 1) * 512],
                        start=(mt == 0),
                        stop=(mt == MT - 1),
                    )

        def emit_pv1(ot_ps, h, mt, exmst):
            """PV for a single head h; ot_ps = [chn0, chn1] psum tiles."""
            hl = h % 2
            for chn in range(N // 512):
                nc.tensor.matmul(
                    ot_ps[chn],
                    lhsT=vaug[:, mt, h, :],
                    rhs=exmst[:, mt, hl, chn * 512 : (chn + 1) * 512],
                    start=(mt == 0),
                    stop=(mt == MT - 1),
                )

        def evict_head(ot_ps, h):
            """copy head h's two PV accumulators (plus den row) to ot_sb."""
            for chn in range(2):
                nc.vector.tensor_copy(
                    ot_sb[:, h, chn * 512 : (chn + 1) * 512],
                    ot_ps[chn][: HEAD_DIM + 1, :],
                )

        def normalize_head(h, rbq):
            """softmax-normalize head h from ot_sb into its otn2 half."""
            hp, hl = divmod(h, 2)
            dn = slice(HEAD_DIM, HEAD_DIM + 1)
            # broadcast den row (partition 64) to partitions 0:64 via a K=1
            # ones matmul: rb_ps[d, n] = ones[d] * den[n]
            rb_ps = rbq.tile(
                [HEAD_DIM, N], F32, tag="rbps", name="rbps", bufs=1
            )
            for chn in range(2):
                nc.tensor.matmul(
                    rb_ps[:, chn * 512 : (chn + 1) * 512],
                    lhsT=ones_sb[HEAD_DIM : HEAD_DIM + 1, :],
                    rhs=ot_sb[dn, h, chn * 512 : (chn + 1) * 512],
                    start=True,
                    stop=True,
                )
            rb = rbp.tile([HEAD_DIM, N], F32, tag="rb", name="rb")
            nc.vector.reciprocal_approx_fast(out=rb, in_=rb_ps)
            if hl == 0:
                nc.vector.tensor_mul(
                    otn2[:HEAD_DIM, hp, :], ot_sb[:HEAD_DIM, h, :], rb
                )
            else:
                tmp = rbp.tile([HEAD_DIM, N], BF16, tag="tmp", name="tmp")
                nc.vector.tensor_mul(tmp, ot_sb[:HEAD_DIM, h, :], rb)
                # partition shift 0:64 -> 64:128 via SBUF-SBUF DMA
                nc.sync.dma_start(out=otn2[HEAD_DIM:P, hp, :], in_=tmp)

        def load_mask(mt):
            mk = maskp.tile([P, N], BF16, tag="mk", name="mk")
            nc.gpsimd.dma_start(out=mk, in_=maskt_d[mt * P : (mt + 1) * P, :])
            return mk

        with tc.tile_pool(name="exmp", bufs=1) as exmp:
            # masked exp(scores) parked per m-tile; one buffer reused across
            # head pairs (WAR: stage-2 rewrites a tile only after its PV read)
            exmst = exmp.tile([P, MT, 2, N], BF16)

            with tc.tile_pool(name="wctx", bufs=1) as wctx_pool:
                wkT = wctx_pool.tile([P, CT, E], BF16)
                wvT = wctx_pool.tile([P, CT, E], BF16)
                ctxT = wctx_pool.tile([P, CT, M], BF16)

                with tc.tile_pool(name="qx", bufs=1) as qx_pool:
                    wqT = qx_pool.tile([P, CT, E], BF16)
                    xT = qx_pool.tile([P, CT, N], BF16)
                    # dependency-first DMA order with few, large transfers
                    # (each dispatch pays ~0.6us queue time + flow-control
                    # credits). sync: wq, x full tiles. vector: ctx (cols
                    # 0:512 first for the k-proj critical path). scalar: wk
                    # only, so the exp stream never queues behind DMA
                    # dispatches. gpsimd: wv, wo, then per-m-tile mask.
                    for j in range(CT):
                        nc.sync.dma_start(
                            out=wqT[:, j, :], in_=wqT_d[j * P : (j + 1) * P, :]
                        )
                    for chn in range(2):
                        for j in range(CT):
                            nc.sync.dma_start(
                                out=xT[:, j, chn * 512 : (chn + 1) * 512],
                                in_=xT_d[
                                    j * P : (j + 1) * P,
                                    chn * 512 : (chn + 1) * 512,
                                ],
                            )
                    for j in range(CT):
                        nc.scalar.dma_start(
                            out=wkT[:, j, :], in_=wkT_d[j * P : (j + 1) * P, :]
                        )
                    for j in range(CT):
                        nc.scalar.dma_start(
                            out=ctxT[:, j, 0:512],
                            in_=ctxT_d[j * P : (j + 1) * P, 0:512],
                        )
                    for j in range(CT):
                        nc.scalar.dma_start(
                            out=ctxT[:, j, 512:M],
                            in_=ctxT_d[j * P : (j + 1) * P, 512:M],
                        )
                    for j in range(CT):
                        nc.gpsimd.dma_start(
                            out=wvT[:, j, :], in_=wvT_d[j * P : (j + 1) * P, :]
                        )
                    for t in range(E // P):
                        nc.gpsimd.dma_start(
                            out=woT[:, t, :], in_=woT_d[t * P : (t + 1) * P, :]
                        )

                    # Q projection, chunk-major so the first psum tile only
                    # needs wq + the first x half-tiles
                    with tc.tile_pool(name="ppsA", bufs=3, space="PSUM") as ppsA:
                        for chn in range(N // 512):
                            for et in range(E // P):
                                pq = ppsA.tile([P, 512], F32, tag="pq")
                                for j in range(CT):
                                    nc.tensor.matmul(
                                        pq,
                                        lhsT=wqT[:, j, et * P : (et + 1) * P],
                                        rhs=xT[:, j, chn * 512 : (chn + 1) * 512],
                                        start=(j == 0),
                                        stop=(j == CT - 1),
                                    )
                                nc.vector.tensor_copy(
                                    qT[:, et, chn * 512 : (chn + 1) * 512], pq
                                )

                def emit_kproj(kps, et, chm):
                    pk = kps.tile([P, 512], F32, tag="pk", name="pk")
                    for j in range(CT):
                        nc.tensor.matmul(
                            pk,
                            lhsT=wkT[:, j, et * P : (et + 1) * P],
                            rhs=ctxT[:, j, chm * 512 : (chm + 1) * 512],
                            start=(j == 0),
                            stop=(j == CT - 1),
                        )
                    nc.vector.tensor_copy(
                        kT[:, et, chm * 512 : (chm + 1) * 512], pk
                    )

                # K projection for the first head pair's first chunk must
                # precede stage 1; the rest is folded into stage 1's PE slack.
                # stage 1: scores(heads 0,1) [ACT-bound] + V and K
                # projections interleaved on the otherwise idle PE.
                with (
                    tc.tile_pool(name="sps1", bufs=1, space="PSUM") as sps1,
                    tc.tile_pool(name="vps", bufs=2, space="PSUM") as vps,
                    tc.tile_pool(name="kps", bufs=2, space="PSUM") as kps,
                ):
                    emit_kproj(kps, 0, 0)
                    for mt in range(MT):
                        # keep kT(et0) one chunk ahead of the scores that
                        # consume it; kT(et1) lands before stage 2. Deferred
                        # pacing (chunk c at mt=2c) so a kproj never blocks
                        # the tensor queue on not-yet-arrived ctx columns.
                        if mt % 2 == 0 and mt > 0:
                            et, chm = divmod(mt // 2, M // 512)
                            if et < 2:
                                emit_kproj(kps, et, chm)
                        mk = load_mask(mt)
                        emit_scores(sps1, 2, 0, mt, exmst, mk)
                        pv = vps.tile([P, HPC, HEAD_DIM], F32, tag="pv")
                        for j in range(CT):
                            nc.tensor.matmul(
                                pv,
                                lhsT=ctxT[:, j, mt * P : (mt + 1) * P],
                                rhs=wvT[:, j, :],
                                start=(j == 0),
                                stop=(j == CT - 1),
                            )
                        # single fused eviction of all 4 heads' v columns
                        nc.vector.tensor_copy(
                            vaug[:, mt, :, :HEAD_DIM], pv
                        )

            # stage 2: PV(heads 0,1) interleaved with scores(heads 2,3)
            with tc.tile_pool(name="ops0", bufs=1, space="PSUM") as ops0:
                ot_ps0 = [
                    ops0.tile([P, 512], F32, tag=f"o{i}", name=f"o{i}")
                    for i in range(4)
                ]
                with tc.tile_pool(name="sps2", bufs=1, space="PSUM") as sps2:
                    for mt in range(MT):
                        mk = load_mask(mt)
                        emit_pv(ot_ps0, 0, mt, exmst)
                        emit_scores(sps2, 2, 1, mt, exmst, mk)
                evict_head(ot_ps0[0:2], 0)
                evict_head(ot_ps0[2:4], 1)

            # stage 3: PV head 2 (normalize heads 0,1 in its shadow), PV
            # head 3 (normalize head 2), normalize head 3
            with (
                tc.tile_pool(name="ops1", bufs=1, space="PSUM") as ops1,
                tc.tile_pool(name="rbq", bufs=1, space="PSUM") as rbq,
            ):
                ot_ps2 = [
                    ops1.tile([P, 512], F32, tag=f"p{i}", name=f"p{i}")
                    for i in range(2)
                ]
                for mt in range(MT):
                    emit_pv1(ot_ps2, 2, mt, exmst)
                    if mt == 2:
                        normalize_head(0, rbq)
                    if mt == 8:
                        normalize_head(1, rbq)
                evict_head(ot_ps2, 2)
                ot_ps3 = [
                    ops1.tile([P, 512], F32, tag=f"q{i}", name=f"q{i}")
                    for i in range(2)
                ]
                for mt in range(MT):
                    emit_pv1(ot_ps3, 3, mt, exmst)
                    if mt == 2:
                        normalize_head(2, rbq)
                evict_head(ot_ps3, 3)
                normalize_head(3, rbq)

            if kdbg:
                nc.sync.dma_start(out=otdump_d, in_=ot_sb)

        # ---------- output projection ----------
        # wide 2-bank psum tiles per row-block, one eviction per block
        # alternating between DVE and ACT, DMA alternating rings
        with (
            tc.tile_pool(name="ypsum", bufs=2, space="PSUM") as ypsum,
            tc.tile_pool(name="ypool", bufs=3) as ypool,
        ):
            for nb in range(N // P):
                yp = ypsum.tile([P, 2, 512], F32, tag="yp")
                for oc in range(DIM // 512):
                    for hp in range(E // P):
                        nc.tensor.matmul(
                            yp[:, oc, :],
                            lhsT=otn2[:, hp, nb * P : (nb + 1) * P],
                            rhs=woT[:, hp, oc * 512 : (oc + 1) * 512],
                            start=(hp == 0),
                            stop=(hp == E // P - 1),
                        )
                ys = ypool.tile([P, 2, 512], BF16, tag="ys")
                if nb % 2:
                    nc.vector.tensor_copy(ys, yp)
                else:
                    nc.scalar.copy(ys, yp)
                ring = nc.scalar if nb % 2 else nc.sync
                ring.dma_start(
                    out=y_d[nb * P : (nb + 1) * P, :], in_=ys
                )

    nc.compile()
    return nc


_NC_CACHE = []


def _get_nc():
    if not _NC_CACHE:
        _NC_CACHE.append(build_program())
    return _NC_CACHE[0]


def make_in_maps(x, context, mask, Wq, Wkv, b_kv, Wo):
    bf = ml_dtypes.bfloat16
    x = np.asarray(x, dtype=np.float32)
    context = np.asarray(context, dtype=np.float32)
    mask = np.asarray(mask)
    Wq = np.asarray(Wq, dtype=np.float32)
    Wkv = np.asarray(Wkv, dtype=np.float32)
    Wo = np.asarray(Wo, dtype=np.float32)

    in_maps = []
    for b in range(B):
        xtb = np.ascontiguousarray(x[b].T).astype(bf)
        ctb = np.ascontiguousarray(context[b].T).astype(bf)
        mtb = np.ascontiguousarray(mask[b].T).astype(bf)
        for g in range(NUM_HEADS // HPC):
            sl = slice(E * g, E * (g + 1))
            in_maps.append(
                {
                    "xT": xtb,
                    "ctxT": ctb,
                    "maskt": mtb,
                    "wqT": np.ascontiguousarray(Wq[sl].T).astype(bf),
                    "wkT": np.ascontiguousarray(Wkv[sl].T).astype(bf),
                    "wvT": np.ascontiguousarray(
                        Wkv[DIM + E * g : DIM + E * (g + 1)].T
                    ).astype(bf),
                    "woT": np.ascontiguousarray(Wo[:, sl].T).astype(bf),
                }
            )
    return in_maps


def combine_outputs(ys, b_kv, Wo):
    """ys: list of 8 per-core partial outputs [N, DIM], core order (b, g)."""
    b_v = np.asarray(b_kv, dtype=np.float32)[DIM:]
    ybias = np.asarray(Wo, dtype=np.float32) @ b_v  # [DIM]
    out = np.empty((B, N, DIM), dtype=np.float32)
    G = NUM_HEADS // HPC
    for b in range(B):
        acc = np.asarray(ys[G * b], dtype=np.float32)
        for g in range(1, G):
            acc = acc + np.asarray(ys[G * b + g], dtype=np.float32)
        out[b] = acc + ybias[None, :]
    return out


def kernel(x, context, mask, Wq, Wkv, b_kv, Wo):
    nc = _get_nc()
    in_maps = make_in_maps(x, context, mask, Wq, Wkv, b_kv, Wo)
    res = run_bass_kernel_spmd(nc, in_maps, core_ids=list(range(8)))
    ys = [m["y"] for m in res.results]
    return combine_outputs(ys, b_kv, Wo)


# revision 29
# speedup vs baseline: 1.0891x; 1.0891x over previous
"""CrossAttention Trainium2 kernel (8 NeuronCores, SPMD), bf16 compute.

Sharding: data-parallel over batch B=2, tensor-parallel over the 16 heads in
4 groups of 4 heads -> 8 cores, one (batch, head-group) pair each. Each core
computes its 4 heads' Q/K/V projections, masked softmax cross-attention, and
its partial output projection y_g = softmax(q k^T * scale) v @ Wo[:, g].T.
The host sums the 4 partial outputs per batch (the Wo row-split all-reduce,
done at unshard time) and adds the v-bias term Wo @ b_v, which is constant
across rows and factors out of the attention (softmax rows sum to 1).
The k-bias is dropped entirely: (k+b)^T q adds a per-query constant to the
logits, which softmax over keys is invariant to.

Numerics: inputs are cast to bf16 on the host; every matmul runs bf16 x bf16
with fp32 PSUM accumulation; softmax statistics (denominator, reciprocal,
normalization) stay fp32. Output partials are written bf16 and summed fp32
on the host. End-to-end relative error ~3e-3.

Layout: the PE contracts over the partition dim, so activations and weights
arrive contraction-major (pre-transposed on host); every device DMA is a
plain contiguous row load; no transposes on the device.

Attention is computed scores-transposed: ST[m, n] per head so PV contracts
over m directly; the two heads of a pair sit at partitions 0:64 / 64:128 so
their score matmuls run concurrently as PE row-tiles. The softmax
denominator comes free from a ones-column appended to v. exp() is
unnormalized (|s*scale| < ~4); mask zeros are applied multiplicatively
after exp with one wide (free=2048) DVE multiply per (pair, m-tile).

Stages (PE-dense, ACT overlapped):
  Q proj (chunk-major, DMA-paced) ->
  stage 1: scores+exp+mask heads 0,1 with K and V projections on PE slack
  stage 2: PV heads 0,1 interleaved with scores+exp+mask heads 2,3
  stage 3: PV head 2 (normalize heads 0,1 in its shadow), PV head 3
           (normalize head 2), normalize head 3, output projection.
Normalization: the denominator row is broadcast to 64 partitions with a
K=1 ones matmul on the PE (rhs read from the evicted ot_sb row at
partition 64), then DVE reciprocal + scale; odd heads shift into the upper
partition half via SBUF-SBUF DMA.
"""

import os

import numpy as np
import ml_dtypes

import concourse.bass as bass
import concourse.bacc as bacc
import concourse.mybir as mybir
import concourse.tile as tile
from concourse.bass_utils import run_bass_kernel_spmd

DIM = 1024
HEAD_DIM = 64
NUM_HEADS = 16
SCALE = HEAD_DIM**-0.5
B, N, M = 2, 1024, 2048
HPC = 4  # heads per core
E = HPC * HEAD_DIM  # 256: per-core projection width
P = 128
F32 = mybir.dt.float32
BF16 = mybir.dt.bfloat16
CT = DIM // P  # 8 contraction tiles
MT = M // P  # 16 m tiles


def build_program():
    nc = bacc.Bacc("TRN2", target_bir_lowering=False, debug=False, num_devices=8)

    # all activation/weight shards arrive contraction-major (pre-transposed)
    xT_d = nc.dram_tensor("xT", [DIM, N], BF16, kind="ExternalInput").ap()
    ctxT_d = nc.dram_tensor("ctxT", [DIM, M], BF16, kind="ExternalInput").ap()
    maskt_d = nc.dram_tensor("maskt", [M, N], BF16, kind="ExternalInput").ap()
    wqT_d = nc.dram_tensor("wqT", [DIM, E], BF16, kind="ExternalInput").ap()
    wkT_d = nc.dram_tensor("wkT", [DIM, E], BF16, kind="ExternalInput").ap()
    wvT_d = nc.dram_tensor("wvT", [DIM, E], BF16, kind="ExternalInput").ap()
    woT_d = nc.dram_tensor("woT", [E, DIM], BF16, kind="ExternalInput").ap()
    y_d = nc.dram_tensor("y", [N, DIM], BF16, kind="ExternalOutput").ap()

    kdbg = bool(os.environ.get("KDBG"))
    if kdbg:
        otdump_d = nc.dram_tensor(
            "otdump", [HEAD_DIM + 1, HPC, N], F32, kind="ExternalOutput"
        ).ap()

    Exp = mybir.ActivationFunctionType.Exp

    from contextlib import ExitStack

    with tile.TileContext(nc) as tc, ExitStack() as ctx:
        persist = ctx.enter_context(tc.tile_pool(name="persist", bufs=1))
        qT = persist.tile([P, E // P, N], BF16)
        kT = persist.tile([P, E // P, M], BF16)
        # v columns padded to 128 so the PV stationary is a full-width
        # weight load (enables Fast Weight Load; col 64 = ones for the
        # softmax denominator, cols 65:128 = don't-care)
        vaug = persist.tile([P, MT, HPC, P], BF16)
        woT = persist.tile([P, E // P, DIM], BF16)
        otn2 = persist.tile([P, E // P, N], BF16)
        ones_sb = persist.tile([P, HEAD_DIM], BF16)
        # rows 0:64 unnormalized attention out, row 64 denominator (bf16:
        # the ~0.4% rounding is well within the error budget and halves
        # the SBUF footprint; softmax statistics stay fp32 downstream)
        ot_sb = persist.tile([HEAD_DIM + 1, HPC, N], BF16)

        # ones column: fill everything; v evictions overwrite cols 0:64
        nc.vector.memset(vaug, 1.0)
        nc.vector.memset(ones_sb, 1.0)

        bwork = ctx.enter_context(tc.tile_pool(name="bwork", bufs=2))
        maskp = ctx.enter_context(tc.tile_pool(name="maskp", bufs=2))
        rbp = ctx.enter_context(tc.tile_pool(name="rbp", bufs=1))

        def emit_scores(spool, sbufs, hp, mt, exmst, mk):
            """scores -> exp for head pair hp at m-tile mt (per n-chunk PSUM
            tiles, double-buffered), then one wide masked multiply."""
            ex = bwork.tile([P, 2, N], BF16, tag="ex", name="ex")
            for chn in range(N // 512):
                st = spool.tile(
                    [P, 2, 512], F32, tag="st", name="st", bufs=sbufs
                )
                for hl in range(2):
                    erow = slice(hl * HEAD_DIM, (hl + 1) * HEAD_DIM)
                    nc.tensor.matmul(
                        st[:, hl, :],
                        lhsT=kT[erow, hp, mt * P : (mt + 1) * P],
                        rhs=qT[erow, hp, chn * 512 : (chn + 1) * 512],
                        start=True,
                        stop=True,
                    )
                nc.scalar.activation(
                    ex[:, :, chn * 512 : (chn + 1) * 512], st, Exp,
                    scale=float(SCALE),
                )
            mkc = bass.AP(mk.tensor, mk.offset, [mk.ap[0], [0, 2], mk.ap[1]])
            nc.vector.tensor_mul(exmst[:, mt, :, :], ex, mkc)

        def emit_pv(ot_ps, hp, mt, exmst):
            for hl in range(2):
                h = hp * 2 + hl
                for chn in range(N // 512):
                    nc.tensor.matmul(
                        ot_ps[hl * 2 + chn],
                        lhsT=vaug[:, mt, h, :],
                        rhs=exmst[:, mt, hl, chn * 512 : (chn + 1) * 512],
                        start=(mt == 0),
                        stop=(mt == MT - 1),
                    )

        def emit_pv1(ot_ps, h, mt, exmst):
            """PV for a single head h; ot_ps = [chn0, chn1] psum tiles."""
            hl = h % 2
            for chn in range(N // 512):
                nc.tensor.matmul(
                    ot_ps[chn],
                    lhsT=vaug[:, mt, h, :],
                    rhs=exmst[:, mt, hl, chn * 512 : (chn + 1) * 512],
                    start=(mt == 0),
                    stop=(mt == MT - 1),
                )

        def evict_head(ot_ps, h):
            """copy head h's two PV accumulators (plus den row) to ot_sb."""
            for chn in range(2):
                nc.vector.tensor_copy(
                    ot_sb[:, h, chn * 512 : (chn + 1) * 512],
                    ot_ps[chn][: HEAD_DIM + 1, :],
                )

        def normalize_head(h, rbq):
            """softmax-normalize head h from ot_sb into its otn2 half. The
            denominator row (partition 64) is broadcast to partitions 0:64
            with a K=1 bf16 ones matmul on the PE: rb_ps[d,n] = den[n]."""
            hp, hl = divmod(h, 2)
            dn = slice(HEAD_DIM, HEAD_DIM + 1)
            rb_ps = rbq.tile(
                [HEAD_DIM, N], F32, tag="rbps", name="rbps", bufs=1
            )
            for chn in range(2):
                nc.tensor.matmul(
                    rb_ps[:, chn * 512 : (chn + 1) * 512],
                    lhsT=ones_sb[HEAD_DIM : HEAD_DIM + 1, :],
                    rhs=ot_sb[dn, h, chn * 512 : (chn + 1) * 512],
                    start=True,
                    stop=True,
                )
            rb = rbp.tile([HEAD_DIM, N], F32, tag="rb", name="rb")
            nc.vector.reciprocal_approx_fast(out=rb, in_=rb_ps)
            if hl == 0:
                nc.vector.tensor_mul(
                    otn2[:HEAD_DIM, hp, :], ot_sb[:HEAD_DIM, h, :], rb
                )
            else:
                tmp = rbp.tile([HEAD_DIM, N], BF16, tag="tmp", name="tmp")
                nc.vector.tensor_mul(tmp, ot_sb[:HEAD_DIM, h, :], rb)
                # partition shift 0:64 -> 64:128 via SBUF-SBUF DMA
                nc.sync.dma_start(out=otn2[HEAD_DIM:P, hp, :], in_=tmp)

        maskt_r = maskt_d.rearrange("(mt p) n -> p mt n", p=P)

        def load_mask_group(g):
            """one 1MB DMA covering four m-tiles of the mask (big transfers
            spread over all 16 SDMA engines at ~3x the small-DMA rate)."""
            mkg = maskp.tile([P, 4, N], BF16, tag="mkg", name="mkg")
            nc.gpsimd.dma_start(out=mkg, in_=maskt_r[:, 4 * g : 4 * (g + 1), :])
            return mkg

        with tc.tile_pool(name="exmp", bufs=1) as exmp:
            # masked exp(scores) parked per m-tile; one buffer reused across
            # head pairs (WAR: stage-2 rewrites a tile only after its PV read)
            exmst = exmp.tile([P, MT, 2, N], BF16)

            with tc.tile_pool(name="wctx", bufs=1) as wctx_pool:
                wkT = wctx_pool.tile([P, CT, E], BF16)
                wvT = wctx_pool.tile([P, CT, E], BF16)
                ctxT = wctx_pool.tile([P, CT, M], BF16)

                with tc.tile_pool(name="qx", bufs=1) as qx_pool:
                    wqT = qx_pool.tile([P, CT, E], BF16)
                    xT = qx_pool.tile([P, CT, N], BF16)
                    # few, large DMAs: a single dma_start is split across all
                    # 16 SDMA engines (>=1MB hits ~300GB/s; 64-256KB
                    # transfers are descriptor-dominated at ~100GB/s).
                    # Dependency-first order; sync+scalar are the two HWDGE
                    # rings, gpsimd is the software ring.
                    xT_r = xT_d.rearrange("(j p) n -> p j n", p=P)
                    ctxT_r = ctxT_d.rearrange("(j p) m -> p j m", p=P)
                    nc.sync.dma_start(
                        out=wqT, in_=wqT_d.rearrange("(j p) e -> p j e", p=P)
                    )
                    nc.sync.dma_start(out=xT[:, :, 0:512], in_=xT_r[:, :, 0:512])
                    nc.sync.dma_start(out=xT[:, :, 512:N], in_=xT_r[:, :, 512:N])
                    nc.scalar.dma_start(
                        out=wkT, in_=wkT_d.rearrange("(j p) e -> p j e", p=P)
                    )
                    nc.scalar.dma_start(
                        out=ctxT[:, :, 0:512], in_=ctxT_r[:, :, 0:512]
                    )
                    nc.scalar.dma_start(
                        out=ctxT[:, :, 512:1024], in_=ctxT_r[:, :, 512:1024]
                    )
                    nc.sync.dma_start(
                        out=ctxT[:, :, 1024:M], in_=ctxT_r[:, :, 1024:M]
                    )
                    nc.gpsimd.dma_start(
                        out=wvT, in_=wvT_d.rearrange("(j p) e -> p j e", p=P)
                    )
                    nc.gpsimd.dma_start(
                        out=woT, in_=woT_d.rearrange("(t p) d -> p t d", p=P)
                    )

                    # Q projection, chunk-major so the first psum tile only
                    # needs wq + the first x half-tiles
                    with tc.tile_pool(name="ppsA", bufs=3, space="PSUM") as ppsA:
                        for chn in range(N // 512):
                            for et in range(E // P):
                                pq = ppsA.tile([P, 512], F32, tag="pq")
                                for j in range(CT):
                                    nc.tensor.matmul(
                                        pq,
                                        lhsT=wqT[:, j, et * P : (et + 1) * P],
                                        rhs=xT[:, j, chn * 512 : (chn + 1) * 512],
                                        start=(j == 0),
                                        stop=(j == CT - 1),
                                    )
                                nc.vector.tensor_copy(
                                    qT[:, et, chn * 512 : (chn + 1) * 512], pq
                                )

                def emit_kproj(kps, et, chm):
                    pk = kps.tile([P, 512], F32, tag="pk", name="pk")
                    for j in range(CT):
                        nc.tensor.matmul(
                            pk,
                            lhsT=wkT[:, j, et * P : (et + 1) * P],
                            rhs=ctxT[:, j, chm * 512 : (chm + 1) * 512],
                            start=(j == 0),
                            stop=(j == CT - 1),
                        )
                    nc.vector.tensor_copy(
                        kT[:, et, chm * 512 : (chm + 1) * 512], pk
                    )

                # K projection for the first head pair's first chunk must
                # precede stage 1; the rest is folded into stage 1's PE slack.
                # stage 1: scores(heads 0,1) [ACT-bound] + V and K
                # projections interleaved on the otherwise idle PE.
                with (
                    tc.tile_pool(name="sps1", bufs=1, space="PSUM") as sps1,
                    tc.tile_pool(name="vps", bufs=2, space="PSUM") as vps,
                    tc.tile_pool(name="kps", bufs=2, space="PSUM") as kps,
                ):
                    # kproj pacing: late enough that the tensor queue never
                    # blocks on not-yet-arrived ctx columns, early enough
                    # that kT stays ahead of the scores that consume it
                    # (et0 chunk c feeds scores mts 4c..4c+3; et1 feeds
                    # stage 2).
                    kproj_at = {4: (0, 1), 6: (0, 2), 8: (0, 3),
                                9: (1, 0), 11: (1, 1), 13: (1, 2), 15: (1, 3)}
                    emit_kproj(kps, 0, 0)
                    mkg = None
                    for mt in range(MT):
                        if et_chm := kproj_at.get(mt):
                            emit_kproj(kps, *et_chm)
                        if mt % 4 == 0:
                            mkg = load_mask_group(mt // 4)
                        emit_scores(sps1, 2, 0, mt, exmst, mkg[:, mt % 4, :])
                        pv = vps.tile([P, HPC, HEAD_DIM], F32, tag="pv")
                        for j in range(CT):
                            nc.tensor.matmul(
                                pv,
                                lhsT=ctxT[:, j, mt * P : (mt + 1) * P],
                                rhs=wvT[:, j, :],
                                start=(j == 0),
                                stop=(j == CT - 1),
                            )
                        # single fused eviction of all 4 heads' v columns
                        nc.vector.tensor_copy(
                            vaug[:, mt, :, :HEAD_DIM], pv
                        )

            # stage 2: PV(heads 0,1) interleaved with scores(heads 2,3)
            with tc.tile_pool(name="ops0", bufs=1, space="PSUM") as ops0:
                ot_ps0 = [
                    ops0.tile([P, 512], F32, tag=f"o{i}", name=f"o{i}")
                    for i in range(4)
                ]
                with tc.tile_pool(name="sps2", bufs=1, space="PSUM") as sps2:
                    mkg = None
                    for mt in range(MT):
                        if mt % 4 == 0:
                            mkg = load_mask_group(mt // 4)
                        emit_pv(ot_ps0, 0, mt, exmst)
                        emit_scores(sps2, 2, 1, mt, exmst, mkg[:, mt % 4, :])
                evict_head(ot_ps0[0:2], 0)
                evict_head(ot_ps0[2:4], 1)

            # stage 3: PV head 2 (normalize heads 0,1 in its shadow), PV
            # head 3 (normalize head 2), normalize head 3
            with (
                tc.tile_pool(name="ops1", bufs=1, space="PSUM") as ops1,
                tc.tile_pool(name="rbq", bufs=1, space="PSUM") as rbq,
            ):
                ot_ps2 = [
                    ops1.tile([P, 512], F32, tag=f"p{i}", name=f"p{i}")
                    for i in range(2)
                ]
                for mt in range(MT):
                    emit_pv1(ot_ps2, 2, mt, exmst)
                    if mt == 2:
                        normalize_head(0, rbq)
                    if mt == 8:
                        normalize_head(1, rbq)
                evict_head(ot_ps2, 2)
                ot_ps3 = [
                    ops1.tile([P, 512], F32, tag=f"q{i}", name=f"q{i}")
                    for i in range(2)
                ]
                for mt in range(MT):
                    emit_pv1(ot_ps3, 3, mt, exmst)
                    if mt == 2:
                        normalize_head(2, rbq)
                evict_head(ot_ps3, 3)
                normalize_head(3, rbq)

            if kdbg:
                nc.sync.dma_start(out=otdump_d, in_=ot_sb)

        # ---------- output projection ----------
        # wide 2-bank psum tiles per row-block, one eviction per block
        # alternating between DVE and ACT, DMA alternating rings
        with (
            tc.tile_pool(name="ypsum", bufs=2, space="PSUM") as ypsum,
            tc.tile_pool(name="ypool", bufs=3) as ypool,
        ):
            for nb in range(N // P):
                yp = ypsum.tile([P, 2, 512], F32, tag="yp")
                for oc in range(DIM // 512):
                    for hp in range(E // P):
                        nc.tensor.matmul(
                            yp[:, oc, :],
                            lhsT=otn2[:, hp, nb * P : (nb + 1) * P],
                            rhs=woT[:, hp, oc * 512 : (oc + 1) * 512],
                            start=(hp == 0),
                            stop=(hp == E // P - 1),
                        )
                ys = ypool.tile([P, 2, 512], BF16, tag="ys")
                if nb % 2:
                    nc.vector.tensor_copy(ys, yp)
                else:
                    nc.scalar.copy(ys, yp)
                ring = (nc.sync, nc.scalar, nc.gpsimd)[nb % 3]
                ring.dma_start(
                    out=y_d[nb * P : (nb + 1) * P, :], in_=ys
                )

    nc.compile()
    return nc


_NC_CACHE = []


def _get_nc():
    if not _NC_CACHE:
        _NC_CACHE.append(build_program())
    return _NC_CACHE[0]


def make_in_maps(x, context, mask, Wq, Wkv, b_kv, Wo):
    bf = ml_dtypes.bfloat16
    x = np.asarray(x, dtype=np.float32)
    context = np.asarray(context, dtype=np.float32)
    mask = np.asarray(mask)
    Wq = np.asarray(Wq, dtype=np.float32)
    Wkv = np.asarray(Wkv, dtype=np.float32)
    Wo = np.asarray(Wo, dtype=np.float32)

    in_maps = []
    for b in range(B):
        xtb = np.ascontiguousarray(x[b].T).astype(bf)
        ctb = np.ascontiguousarray(context[b].T).astype(bf)
        mtb = np.ascontiguousarray(mask[b].T).astype(bf)
        for g in range(NUM_HEADS // HPC):
            sl = slice(E * g, E * (g + 1))
            in_maps.append(
                {
                    "xT": xtb,
                    "ctxT": ctb,
                    "maskt": mtb,
                    "wqT": np.ascontiguousarray(Wq[sl].T).astype(bf),
                    "wkT": np.ascontiguousarray(Wkv[sl].T).astype(bf),
                    "wvT": np.ascontiguousarray(
                        Wkv[DIM + E * g : DIM + E * (g + 1)].T
                    ).astype(bf),
                    "woT": np.ascontiguousarray(Wo[:, sl].T).astype(bf),
                }
            )
    return in_maps


def combine_outputs(ys, b_kv, Wo):
    """ys: list of 8 per-core partial outputs [N, DIM], core order (b, g)."""
    b_v = np.asarray(b_kv, dtype=np.float32)[DIM:]
    ybias = np.asarray(Wo, dtype=np.float32) @ b_v  # [DIM]
    out = np.empty((B, N, DIM), dtype=np.float32)
    G = NUM_HEADS // HPC
    for b in range(B):
        acc = np.asarray(ys[G * b], dtype=np.float32)
        for g in range(1, G):
            acc = acc + np.asarray(ys[G * b + g], dtype=np.float32)
        out[b] = acc + ybias[None, :]
    return out


def kernel(x, context, mask, Wq, Wkv, b_kv, Wo):
    nc = _get_nc()
    in_maps = make_in_maps(x, context, mask, Wq, Wkv, b_kv, Wo)
    res = run_bass_kernel_spmd(nc, in_maps, core_ids=list(range(8)))
    ys = [m["y"] for m in res.results]
    return combine_outputs(ys, b_kv, Wo)
